# revision 2
# baseline (speedup 1.0000x reference)
"""Trainium2 Bass kernel for nn_NumDualDescriptorAB (sliding-window descriptor).

Reference computation:
    X = vec_seq @ M.T                       # [S, m]
    T[w] = mean_{r<rank} X[w+r]             # sliding window mean, W = S-rank+1
    j = w % L
    scalar[w] = Bbasis[j] . T[w]
    out[w]    = Acoeff.T[j] * scalar[w]

Algebraic rewrite (matmul is linear, dot distributes over the window sum):
    C = Bbasis @ M / rank                   # [L, m]  tiny - host precompute
    P[w] = sum_{r<rank} vec_seq[w+r]        # window *sum* of raw input rows
    scalar[w] = C[j] . P[w]
    out[w]    = Acoeff.T[j] * scalar[w]

v2 design (DMA-roofline targeted; baseline was DMA-bound at 89% with
256KB transfers and a 178us DVE tail):
  - bf16 input AND output HBM traffic (33MB/core vs 67MB) - tolerance is
    2e-2, bf16 end-to-end lands ~3e-3.
  - DMA batched into [128, BC, 512] blocks (BC=16 -> 2MB per transfer)
    via transposed access patterns; HBM side stays fully contiguous.
  - PE: banded 0/1-weight matmuls compute P per 128-window tile.
  - DVE: ONE fused pass per tile - scalar_tensor_tensor computes
    (P * C[j]) elementwise AND accum_out the free-axis sum -> scalar.
  - Broadcast out[w] = scalar[w] * AT[j] alternates ACT / GPSIMD so
    neither engine becomes the bottleneck.
Sharded across 8 cores along the window axis; halo handled host-side by
overlapping shards (no collectives).
"""

import numpy as np

import concourse.bacc as bacc
import concourse.bass as bass  # noqa: F401
import concourse.mybir as mybir
import concourse.tile as tile
from concourse.bass_utils import run_bass_kernel_spmd

N_CORES = 8
M_DIM = 512
L_DIM = 512
SEQ = 131072
CHUNK = 128  # rows per chunk == windows per tile
BC = 16  # chunks per DMA block (2MB bf16 per block transfer)
PF = 2  # block prefetch depth

_NC_CACHE = {}
_LAST_RESULTS = None  # BassKernelResults of the most recent run (for test.py)


def build_nc(nblocks: int, rank: int) -> bass.Bass:
    f32 = mybir.dt.float32
    bf16 = mybir.dt.bfloat16
    halo = rank - 1
    ntiles = nblocks * BC

    mult = mybir.AluOpType.mult
    copy_f = mybir.ActivationFunctionType.Copy

    nc = bacc.Bacc()
    v_d = nc.dram_tensor("v", [ntiles + 1, CHUNK, M_DIM], bf16, kind="ExternalInput")
    c_d = nc.dram_tensor("cmat", [4, CHUNK, M_DIM], f32, kind="ExternalInput")
    a_d = nc.dram_tensor("amat", [4, CHUNK, M_DIM], bf16, kind="ExternalInput")
    w1_d = nc.dram_tensor("w1", [CHUNK, CHUNK], bf16, kind="ExternalInput")
    if halo > 0:
        w2_d = nc.dram_tensor("w2", [halo, CHUNK], bf16, kind="ExternalInput")
    o_d = nc.dram_tensor("o", [nblocks, BC, CHUNK, M_DIM], bf16, kind="ExternalOutput")

    with tile.TileContext(nc) as tc:
        with (
            tc.tile_pool(name="consts", bufs=1) as consts,
            tc.tile_pool(name="blocks", bufs=PF + 1) as blocks,
            tc.tile_pool(name="outs", bufs=2) as outs,
            tc.tile_pool(name="psump", bufs=8, space="PSUM") as psump,
            tc.tile_pool(name="work", bufs=8) as work,
        ):
            c4 = consts.tile([CHUNK, 4, M_DIM], f32, tag="c4")
            nc.sync.dma_start(out=c4, in_=c_d[:].transpose([1, 0, 2]))
            a4 = consts.tile([CHUNK, 4, M_DIM], bf16, tag="a4")
            nc.sync.dma_start(out=a4, in_=a_d[:].transpose([1, 0, 2]))
            w1t = consts.tile([CHUNK, CHUNK], bf16, tag="w1")
            nc.sync.dma_start(out=w1t, in_=w1_d[:])
            if halo > 0:
                w2t = consts.tile([halo, CHUNK], bf16, tag="w2")
                nc.sync.dma_start(out=w2t, in_=w2_d[:])
            # final halo chunk (row block ntiles)
            tailt = consts.tile([CHUNK, M_DIM], bf16, tag="tail")
            nc.sync.dma_start(out=tailt, in_=v_d[ntiles])

            def load_block(b):
                vt = blocks.tile([CHUNK, BC, M_DIM], bf16, tag="vt")
                nc.sync.dma_start(
                    out=vt,
                    in_=v_d[b * BC : (b + 1) * BC].transpose([1, 0, 2]),
                )
                return vt

            vts = {b: load_block(b) for b in range(min(PF, nblocks))}
            for b in range(nblocks):
                if b + PF < nblocks:
                    vts[b + PF] = load_block(b + PF)
                vt = vts[b]
                ot = outs.tile([CHUNK, BC, M_DIM], bf16, tag="ot")
                for c in range(BC):
                    t = b * BC + c
                    ph = t % 4
                    ps = psump.tile([CHUNK, M_DIM], f32, tag="ps")
                    nc.tensor.matmul(
                        ps, w1t, vt[:, c, :], start=True, stop=(halo == 0)
                    )
                    if halo > 0:
                        if c < BC - 1:
                            nxt = vt[:, c + 1, :]
                        elif b < nblocks - 1:
                            nxt = vts[b + 1][:, 0, :]
                        else:
                            nxt = tailt[:]
                        nc.tensor.matmul(
                            ps, w2t, nxt[0:halo, :], start=False, stop=True
                        )
                    sc = work.tile([CHUNK, M_DIM], bf16, tag="sc")
                    s = work.tile([CHUNK, 1], f32, tag="s")
                    nc.vector.scalar_tensor_tensor(
                        out=sc,
                        in0=ps,
                        scalar=1.0,
                        in1=c4[:, ph, :],
                        op0=mult,
                        op1=mult,
                        accum_out=s,
                    )
                    if c % 2 == 0:
                        nc.scalar.activation(
                            out=ot[:, c, :], in_=a4[:, ph, :], func=copy_f, scale=s
                        )
                    else:
                        nc.gpsimd.tensor_scalar(
                            out=ot[:, c, :],
                            in0=a4[:, ph, :],
                            scalar1=s,
                            scalar2=None,
                            op0=mult,
                        )
                nc.sync.dma_start(out=o_d[b].transpose([1, 0, 2]), in_=ot)
                del vts[b]

    nc.finalize()
    return nc


def _get_nc(nblocks: int, rank: int) -> bass.Bass:
    key = (nblocks, rank)
    if key not in _NC_CACHE:
        _NC_CACHE[key] = build_nc(nblocks, rank)
    return _NC_CACHE[key]


def make_band_weights(rank: int):
    """W1[k,w]=1 iff row k of the chunk is inside window w (w<=k<=w+rank-1);
    W2[k,w]=1 iff row k of the *next* chunk is inside window w."""
    w1 = np.zeros((CHUNK, CHUNK), dtype=np.float32)
    for k in range(CHUNK):
        w1[k, max(0, k - (rank - 1)) : k + 1] = 1
    halo = rank - 1
    w2 = np.zeros((max(halo, 1), CHUNK), dtype=np.float32)
    for k in range(halo):
        w2[k, CHUNK - halo + k :] = 1
    return w1, w2


def kernel(vec_seq, M, Acoeff, Bbasis, rank):
    global _LAST_RESULTS
    import ml_dtypes

    bf = ml_dtypes.bfloat16
    vec_seq = np.asarray(vec_seq, dtype=np.float32)
    M = np.asarray(M, dtype=np.float32)
    Acoeff = np.asarray(Acoeff, dtype=np.float32)
    Bbasis = np.asarray(Bbasis, dtype=np.float32)
    r = int(rank)
    S, m = vec_seq.shape
    assert m == M_DIM and Bbasis.shape[0] == L_DIM
    assert 1 <= r <= CHUNK

    W = S - r + 1  # number of windows
    # Per-core window count, padded to a multiple of the block size.
    nblocks = -(-W // (N_CORES * CHUNK * BC))
    ntiles = nblocks * BC
    nw = ntiles * CHUNK
    nrows = (ntiles + 1) * CHUNK

    vec_bf = np.ascontiguousarray(vec_seq).astype(bf)

    # Host-side parameter precompute (tiny: 512^3 matmul). The 1/rank
    # window-mean scale is folded into C.
    C = ((Bbasis.astype(np.float64) @ M.astype(np.float64)) / r).astype(np.float32)
    AT = np.ascontiguousarray(Acoeff.T).astype(np.float32)
    # Tile t uses basis rows j = (128*t .. 128*t+127) % 512 -> phase t%4.
    c4 = np.ascontiguousarray(C.reshape(4, CHUNK, M_DIM))
    a4 = np.ascontiguousarray(AT.reshape(4, CHUNK, M_DIM)).astype(bf)

    w1, w2 = make_band_weights(r)
    w1 = w1.astype(bf)
    w2 = w2.astype(bf)

    nc = _get_nc(nblocks, r)

    in_maps = []
    for k in range(N_CORES):
        lo = k * nw
        hi = min(S, lo + nrows)
        sh = np.zeros((nrows, M_DIM), dtype=bf)
        if hi > lo:
            sh[: hi - lo] = vec_bf[lo:hi]
        im = {
            "v": sh.reshape(ntiles + 1, CHUNK, M_DIM),
            "cmat": c4,
            "amat": a4,
            "w1": w1,
        }
        if r > 1:
            im["w2"] = w2
        in_maps.append(im)

    res = run_bass_kernel_spmd(nc, in_maps, core_ids=list(range(N_CORES)))
    _LAST_RESULTS = res
    out = np.concatenate(
        [res.results[k]["o"].reshape(nw, M_DIM) for k in range(N_CORES)], axis=0
    )
    return np.ascontiguousarray(out[:W].astype(np.float32))


# revision 3
# speedup vs baseline: 2.5526x; 2.5526x over previous
"""Trainium2 Bass kernel for nn_NumDualDescriptorAB (sliding-window descriptor).

Reference computation:
    X = vec_seq @ M.T                       # [S, m]
    T[w] = mean_{r<rank} X[w+r]             # sliding window mean, W = S-rank+1
    j = w % L
    scalar[w] = Bbasis[j] . T[w]
    out[w]    = Acoeff.T[j] * scalar[w]

Algebraic rewrite (matmul is linear, dot distributes over the window sum):
    C = Bbasis @ M / rank                   # [L, m]  tiny - host precompute
    P[w] = sum_{r<rank} vec_seq[w+r]        # window *sum* of raw input rows
    scalar[w] = C[j] . P[w]
    out[w]    = Acoeff.T[j] * scalar[w]

v2 design (DMA-roofline targeted; baseline was DMA-bound at 89% with
256KB transfers and a 178us DVE tail):
  - bf16 input AND output HBM traffic (33MB/core vs 67MB) - tolerance is
    2e-2, bf16 end-to-end lands ~3e-3.
  - DMA batched into [128, BC, 512] blocks (BC=16 -> 2MB per transfer)
    via transposed access patterns; HBM side stays fully contiguous.
  - PE: banded 0/1-weight matmuls compute P per 128-window tile.
  - DVE: ONE fused pass per tile - scalar_tensor_tensor computes
    (P * C[j]) elementwise AND accum_out the free-axis sum -> scalar.
  - Broadcast out[w] = scalar[w] * AT[j] alternates ACT / GPSIMD so
    neither engine becomes the bottleneck.
Sharded across 8 cores along the window axis; halo handled host-side by
overlapping shards (no collectives).
"""

import numpy as np

import concourse.bacc as bacc
import concourse.bass as bass  # noqa: F401
import concourse.mybir as mybir
import concourse.tile as tile
from concourse.bass_utils import run_bass_kernel_spmd

N_CORES = 8
M_DIM = 512
L_DIM = 512
SEQ = 131072
CHUNK = 128  # rows per chunk == windows per tile
BC = 16  # chunks per DMA block (2MB bf16 per block transfer)
PF = 2  # block prefetch depth

_NC_CACHE = {}
_LAST_RESULTS = None  # BassKernelResults of the most recent run (for test.py)


def build_nc(nblocks: int, rank: int) -> bass.Bass:
    f32 = mybir.dt.float32
    bf16 = mybir.dt.bfloat16
    halo = rank - 1
    ntiles = nblocks * BC

    mult = mybir.AluOpType.mult
    copy_f = mybir.ActivationFunctionType.Copy

    nc = bacc.Bacc()
    v_d = nc.dram_tensor("v", [ntiles + 1, CHUNK, M_DIM], bf16, kind="ExternalInput")
    c_d = nc.dram_tensor("cmat", [4, CHUNK, M_DIM], f32, kind="ExternalInput")
    a_d = nc.dram_tensor("amat", [4, CHUNK, M_DIM], bf16, kind="ExternalInput")
    w1_d = nc.dram_tensor("w1", [CHUNK, CHUNK], bf16, kind="ExternalInput")
    if halo > 0:
        w2_d = nc.dram_tensor("w2", [halo, CHUNK], bf16, kind="ExternalInput")
    o_d = nc.dram_tensor("o", [nblocks, BC, CHUNK, M_DIM], bf16, kind="ExternalOutput")

    with tile.TileContext(nc) as tc:
        with (
            tc.tile_pool(name="consts", bufs=1) as consts,
            tc.tile_pool(name="blocks", bufs=PF + 1) as blocks,
            tc.tile_pool(name="outs", bufs=2) as outs,
            tc.tile_pool(name="psump", bufs=8, space="PSUM") as psump,
            tc.tile_pool(name="work", bufs=8) as work,
        ):
            c4 = consts.tile([CHUNK, 4, M_DIM], f32, tag="c4")
            nc.sync.dma_start(out=c4, in_=c_d[:].transpose([1, 0, 2]))
            a4 = consts.tile([CHUNK, 4, M_DIM], bf16, tag="a4")
            nc.sync.dma_start(out=a4, in_=a_d[:].transpose([1, 0, 2]))
            w1t = consts.tile([CHUNK, CHUNK], bf16, tag="w1")
            nc.sync.dma_start(out=w1t, in_=w1_d[:])
            if halo > 0:
                w2t = consts.tile([halo, CHUNK], bf16, tag="w2")
                nc.sync.dma_start(out=w2t, in_=w2_d[:])
            # final halo chunk (row block ntiles)
            tailt = consts.tile([CHUNK, M_DIM], bf16, tag="tail")
            nc.sync.dma_start(out=tailt, in_=v_d[ntiles])

            def load_block(b):
                vt = blocks.tile([CHUNK, BC, M_DIM], bf16, tag="vt")
                nc.sync.dma_start(
                    out=vt,
                    in_=v_d[b * BC : (b + 1) * BC].transpose([1, 0, 2]),
                )
                return vt

            vts = {b: load_block(b) for b in range(min(PF, nblocks))}
            for b in range(nblocks):
                if b + PF < nblocks:
                    vts[b + PF] = load_block(b + PF)
                vt = vts[b]
                ot = outs.tile([CHUNK, BC, M_DIM], bf16, tag="ot")
                for c in range(BC):
                    t = b * BC + c
                    ph = t % 4
                    ps = psump.tile([CHUNK, M_DIM], f32, tag="ps")
                    nc.tensor.matmul(
                        ps, w1t, vt[:, c, :], start=True, stop=(halo == 0)
                    )
                    if halo > 0:
                        if c < BC - 1:
                            nxt = vt[:, c + 1, :]
                        elif b < nblocks - 1:
                            nxt = vts[b + 1][:, 0, :]
                        else:
                            nxt = tailt[:]
                        nc.tensor.matmul(
                            ps, w2t, nxt[0:halo, :], start=False, stop=True
                        )
                    sc = work.tile([CHUNK, M_DIM], bf16, tag="sc")
                    s = work.tile([CHUNK, 1], f32, tag="s")
                    nc.vector.scalar_tensor_tensor(
                        out=sc,
                        in0=ps,
                        scalar=1.0,
                        in1=c4[:, ph, :],
                        op0=mult,
                        op1=mult,
                        accum_out=s,
                    )
                    if t % 7 < 4:
                        nc.scalar.activation(
                            out=ot[:, c, :], in_=a4[:, ph, :], func=copy_f, scale=s
                        )
                    else:
                        # gpsimd tensor_scalar w/ AP scalar measured 7.5us (!);
                        # a stride-0-broadcast tensor_tensor is ~1.1us.
                        nc.gpsimd.tensor_tensor(
                            ot[:, c, :],
                            a4[:, ph, :],
                            s.broadcast_to([CHUNK, M_DIM]),
                            mult,
                        )
                nc.sync.dma_start(out=o_d[b].transpose([1, 0, 2]), in_=ot)
                del vts[b]

    nc.finalize()
    return nc


def _get_nc(nblocks: int, rank: int) -> bass.Bass:
    key = (nblocks, rank)
    if key not in _NC_CACHE:
        _NC_CACHE[key] = build_nc(nblocks, rank)
    return _NC_CACHE[key]


def make_band_weights(rank: int):
    """W1[k,w]=1 iff row k of the chunk is inside window w (w<=k<=w+rank-1);
    W2[k,w]=1 iff row k of the *next* chunk is inside window w."""
    w1 = np.zeros((CHUNK, CHUNK), dtype=np.float32)
    for k in range(CHUNK):
        w1[k, max(0, k - (rank - 1)) : k + 1] = 1
    halo = rank - 1
    w2 = np.zeros((max(halo, 1), CHUNK), dtype=np.float32)
    for k in range(halo):
        w2[k, CHUNK - halo + k :] = 1
    return w1, w2


def kernel(vec_seq, M, Acoeff, Bbasis, rank):
    global _LAST_RESULTS
    import ml_dtypes

    bf = ml_dtypes.bfloat16
    vec_seq = np.asarray(vec_seq, dtype=np.float32)
    M = np.asarray(M, dtype=np.float32)
    Acoeff = np.asarray(Acoeff, dtype=np.float32)
    Bbasis = np.asarray(Bbasis, dtype=np.float32)
    r = int(rank)
    S, m = vec_seq.shape
    assert m == M_DIM and Bbasis.shape[0] == L_DIM
    assert 1 <= r <= CHUNK

    W = S - r + 1  # number of windows
    # Per-core window count, padded to a multiple of the block size.
    nblocks = -(-W // (N_CORES * CHUNK * BC))
    ntiles = nblocks * BC
    nw = ntiles * CHUNK
    nrows = (ntiles + 1) * CHUNK

    vec_bf = np.ascontiguousarray(vec_seq).astype(bf)

    # Host-side parameter precompute (tiny: 512^3 matmul). The 1/rank
    # window-mean scale is folded into C.
    C = ((Bbasis.astype(np.float64) @ M.astype(np.float64)) / r).astype(np.float32)
    AT = np.ascontiguousarray(Acoeff.T).astype(np.float32)
    # Tile t uses basis rows j = (128*t .. 128*t+127) % 512 -> phase t%4.
    c4 = np.ascontiguousarray(C.reshape(4, CHUNK, M_DIM))
    a4 = np.ascontiguousarray(AT.reshape(4, CHUNK, M_DIM)).astype(bf)

    w1, w2 = make_band_weights(r)
    w1 = w1.astype(bf)
    w2 = w2.astype(bf)

    nc = _get_nc(nblocks, r)

    in_maps = []
    for k in range(N_CORES):
        lo = k * nw
        hi = min(S, lo + nrows)
        sh = np.zeros((nrows, M_DIM), dtype=bf)
        if hi > lo:
            sh[: hi - lo] = vec_bf[lo:hi]
        im = {
            "v": sh.reshape(ntiles + 1, CHUNK, M_DIM),
            "cmat": c4,
            "amat": a4,
            "w1": w1,
        }
        if r > 1:
            im["w2"] = w2
        in_maps.append(im)

    res = run_bass_kernel_spmd(nc, in_maps, core_ids=list(range(N_CORES)))
    _LAST_RESULTS = res
    out = np.concatenate(
        [res.results[k]["o"].reshape(nw, M_DIM) for k in range(N_CORES)], axis=0
    )
    return np.ascontiguousarray(out[:W].astype(np.float32))


# revision 4
# speedup vs baseline: 3.0790x; 1.2062x over previous
"""Trainium2 Bass kernel for nn_NumDualDescriptorAB (sliding-window descriptor).

Reference computation:
    X = vec_seq @ M.T                       # [S, m]
    T[w] = mean_{r<rank} X[w+r]             # sliding window mean, W = S-rank+1
    j = w % L
    scalar[w] = Bbasis[j] . T[w]
    out[w]    = Acoeff.T[j] * scalar[w]

Algebraic rewrite (matmul is linear, dot distributes over the window sum):
    C = Bbasis @ M / rank                   # [L, m]  tiny - host precompute
    P[w] = sum_{r<rank} vec_seq[w+r]        # window *sum* of raw input rows
    scalar[w] = C[j] . P[w]
    out[w]    = Acoeff.T[j] * scalar[w]

v2 design (DMA-roofline targeted; baseline was DMA-bound at 89% with
256KB transfers and a 178us DVE tail):
  - bf16 input AND output HBM traffic (33MB/core vs 67MB) - tolerance is
    2e-2, bf16 end-to-end lands ~3e-3.
  - DMA batched into [128, BC, 512] blocks (BC=16 -> 2MB per transfer)
    via transposed access patterns; HBM side stays fully contiguous.
  - PE: banded 0/1-weight matmuls compute P per 128-window tile.
  - DVE: ONE fused pass per tile - scalar_tensor_tensor computes
    (P * C[j]) elementwise AND accum_out the free-axis sum -> scalar.
  - Broadcast out[w] = scalar[w] * AT[j] alternates ACT / GPSIMD so
    neither engine becomes the bottleneck.
Sharded across 8 cores along the window axis; halo handled host-side by
overlapping shards (no collectives).
"""

import numpy as np

import concourse.bacc as bacc
import concourse.bass as bass  # noqa: F401
import concourse.mybir as mybir
import concourse.tile as tile
from concourse.bass_utils import run_bass_kernel_spmd

N_CORES = 8
M_DIM = 512
L_DIM = 512
SEQ = 131072
CHUNK = 128  # rows per chunk == windows per tile
BC = 16  # chunks per DMA block (2MB bf16 per block transfer)
PF = 2  # block prefetch depth

_NC_CACHE = {}
_LAST_RESULTS = None  # BassKernelResults of the most recent run (for test.py)


def build_nc(nblocks: int, rank: int) -> bass.Bass:
    f32 = mybir.dt.float32
    bf16 = mybir.dt.bfloat16
    halo = rank - 1
    ntiles = nblocks * BC

    mult = mybir.AluOpType.mult
    copy_f = mybir.ActivationFunctionType.Copy

    nc = bacc.Bacc()
    v_d = nc.dram_tensor("v", [ntiles + 1, CHUNK, M_DIM], bf16, kind="ExternalInput")
    c_d = nc.dram_tensor("cmat", [4, CHUNK, M_DIM], f32, kind="ExternalInput")
    a_d = nc.dram_tensor("amat", [4, CHUNK, M_DIM], bf16, kind="ExternalInput")
    w1_d = nc.dram_tensor("w1", [CHUNK, CHUNK], bf16, kind="ExternalInput")
    if halo > 0:
        w2_d = nc.dram_tensor("w2", [halo, CHUNK], bf16, kind="ExternalInput")
    o_d = nc.dram_tensor("o", [nblocks, BC, CHUNK, M_DIM], bf16, kind="ExternalOutput")

    with tile.TileContext(nc) as tc:
        with (
            tc.tile_pool(name="consts", bufs=1) as consts,
            tc.tile_pool(name="blocks", bufs=PF + 1) as blocks,
            tc.tile_pool(name="outs", bufs=2) as outs,
            tc.tile_pool(name="psump", bufs=8, space="PSUM") as psump,
            tc.tile_pool(name="work", bufs=8) as work,
        ):
            c4 = consts.tile([CHUNK, 4, M_DIM], f32, tag="c4")
            nc.sync.dma_start(out=c4, in_=c_d[:].transpose([1, 0, 2]))
            a4 = consts.tile([CHUNK, 4, M_DIM], bf16, tag="a4")
            nc.sync.dma_start(out=a4, in_=a_d[:].transpose([1, 0, 2]))
            w1t = consts.tile([CHUNK, CHUNK], bf16, tag="w1")
            nc.sync.dma_start(out=w1t, in_=w1_d[:])
            if halo > 0:
                w2t = consts.tile([halo, CHUNK], bf16, tag="w2")
                nc.sync.dma_start(out=w2t, in_=w2_d[:])
            # final halo chunk (row block ntiles)
            tailt = consts.tile([CHUNK, M_DIM], bf16, tag="tail")
            nc.sync.dma_start(out=tailt, in_=v_d[ntiles])

            def load_block(b):
                vt = blocks.tile([CHUNK, BC, M_DIM], bf16, tag="vt")
                nc.sync.dma_start(
                    out=vt,
                    in_=v_d[b * BC : (b + 1) * BC].transpose([1, 0, 2]),
                )
                return vt

            vts = {b: load_block(b) for b in range(min(PF, nblocks))}
            for b in range(nblocks):
                if b + PF < nblocks:
                    vts[b + PF] = load_block(b + PF)
                vt = vts[b]
                ot = outs.tile([CHUNK, BC, M_DIM], bf16, tag="ot")
                # Half-blocks of H tiles: a dense run of H same-weight W1
                # matmuls, then per-tile W2 + fused dot + broadcast. Keeps
                # the PE stream dense (HAM stays warm) without alternating
                # LDWEIGHTS on every matmul, and caps PSUM pressure at
                # H held + H draining banks.
                H = 4
                for h in range(BC // H):
                    pss = []
                    for c in range(h * H, h * H + H):
                        ps = psump.tile([CHUNK, M_DIM], f32, tag="ps")
                        pss.append(ps)
                        nc.tensor.matmul(
                            ps, w1t, vt[:, c, :], start=True, stop=(halo == 0)
                        )
                    for c in range(h * H, h * H + H):
                        t = b * BC + c
                        ph = t % 4
                        ps = pss[c - h * H]
                        if halo > 0:
                            if c < BC - 1:
                                nxt = vt[:, c + 1, :]
                            elif b < nblocks - 1:
                                nxt = vts[b + 1][:, 0, :]
                            else:
                                nxt = tailt[:]
                            nc.tensor.matmul(
                                ps, w2t, nxt[0:halo, :], start=False, stop=True
                            )
                        sc = work.tile([CHUNK, M_DIM], bf16, tag="sc")
                        s = work.tile([CHUNK, 1], f32, tag="s")
                        nc.vector.scalar_tensor_tensor(
                            out=sc,
                            in0=ps,
                            scalar=1.0,
                            in1=c4[:, ph, :],
                            op0=mult,
                            op1=mult,
                            accum_out=s,
                        )
                        if t % 7 < 4:
                            nc.scalar.activation(
                                out=ot[:, c, :], in_=a4[:, ph, :], func=copy_f, scale=s
                            )
                        else:
                            # gpsimd tensor_scalar w/ AP scalar measured 7.5us;
                            # a stride-0-broadcast tensor_tensor is ~1.26us.
                            nc.gpsimd.tensor_tensor(
                                ot[:, c, :],
                                a4[:, ph, :],
                                s.broadcast_to([CHUNK, M_DIM]),
                                mult,
                            )
                nc.sync.dma_start(out=o_d[b].transpose([1, 0, 2]), in_=ot)
                del vts[b]

    nc.finalize()
    return nc


def _get_nc(nblocks: int, rank: int) -> bass.Bass:
    key = (nblocks, rank)
    if key not in _NC_CACHE:
        _NC_CACHE[key] = build_nc(nblocks, rank)
    return _NC_CACHE[key]


def make_band_weights(rank: int):
    """W1[k,w]=1 iff row k of the chunk is inside window w (w<=k<=w+rank-1);
    W2[k,w]=1 iff row k of the *next* chunk is inside window w."""
    w1 = np.zeros((CHUNK, CHUNK), dtype=np.float32)
    for k in range(CHUNK):
        w1[k, max(0, k - (rank - 1)) : k + 1] = 1
    halo = rank - 1
    w2 = np.zeros((max(halo, 1), CHUNK), dtype=np.float32)
    for k in range(halo):
        w2[k, CHUNK - halo + k :] = 1
    return w1, w2


def kernel(vec_seq, M, Acoeff, Bbasis, rank):
    global _LAST_RESULTS
    import ml_dtypes

    bf = ml_dtypes.bfloat16
    vec_seq = np.asarray(vec_seq, dtype=np.float32)
    M = np.asarray(M, dtype=np.float32)
    Acoeff = np.asarray(Acoeff, dtype=np.float32)
    Bbasis = np.asarray(Bbasis, dtype=np.float32)
    r = int(rank)
    S, m = vec_seq.shape
    assert m == M_DIM and Bbasis.shape[0] == L_DIM
    assert 1 <= r <= CHUNK

    W = S - r + 1  # number of windows
    # Per-core window count, padded to a multiple of the block size.
    nblocks = -(-W // (N_CORES * CHUNK * BC))
    ntiles = nblocks * BC
    nw = ntiles * CHUNK
    nrows = (ntiles + 1) * CHUNK

    vec_bf = np.ascontiguousarray(vec_seq).astype(bf)

    # Host-side parameter precompute (tiny: 512^3 matmul). The 1/rank
    # window-mean scale is folded into C.
    C = ((Bbasis.astype(np.float64) @ M.astype(np.float64)) / r).astype(np.float32)
    AT = np.ascontiguousarray(Acoeff.T).astype(np.float32)
    # Tile t uses basis rows j = (128*t .. 128*t+127) % 512 -> phase t%4.
    c4 = np.ascontiguousarray(C.reshape(4, CHUNK, M_DIM))
    a4 = np.ascontiguousarray(AT.reshape(4, CHUNK, M_DIM)).astype(bf)

    w1, w2 = make_band_weights(r)
    w1 = w1.astype(bf)
    w2 = w2.astype(bf)

    nc = _get_nc(nblocks, r)

    in_maps = []
    for k in range(N_CORES):
        lo = k * nw
        hi = min(S, lo + nrows)
        sh = np.zeros((nrows, M_DIM), dtype=bf)
        if hi > lo:
            sh[: hi - lo] = vec_bf[lo:hi]
        im = {
            "v": sh.reshape(ntiles + 1, CHUNK, M_DIM),
            "cmat": c4,
            "amat": a4,
            "w1": w1,
        }
        if r > 1:
            im["w2"] = w2
        in_maps.append(im)

    res = run_bass_kernel_spmd(nc, in_maps, core_ids=list(range(N_CORES)))
    _LAST_RESULTS = res
    out = np.concatenate(
        [res.results[k]["o"].reshape(nw, M_DIM) for k in range(N_CORES)], axis=0
    )
    return np.ascontiguousarray(out[:W].astype(np.float32))


# revision 5
# speedup vs baseline: 4.7156x; 1.5315x over previous
"""Trainium2 Bass kernel for nn_NumDualDescriptorAB (sliding-window descriptor).

Reference computation:
    X = vec_seq @ M.T                       # [S, m]
    T[w] = mean_{r<rank} X[w+r]             # sliding window mean, W = S-rank+1
    j = w % L
    scalar[w] = Bbasis[j] . T[w]
    out[w]    = Acoeff.T[j] * scalar[w]

Algebraic rewrite (matmul is linear, dot distributes over the window sum):
    C = Bbasis @ M / rank                   # [L, m]  tiny - host precompute
    scalar[w] = sum_{r} C[w%L] . v[w+r]
    out[w]    = Acoeff.T[w%L] * scalar[w]

v5 dataflow ("transposed-V"): the input is uploaded TRANSPOSED (host-side
transpose is free), so the PE contracts over the feature dim directly:

    UT[j, k] = C[ph*128+j] . v[row k]       # per 128-window tile, phase ph
    scalar[w] = sum_{k=w..w+r-1} UT[w, k]   # banded free-axis sum: ONE fused
                                            # DVE scalar_tensor_tensor with a
                                            # constant 0/1 band mask, FD=143
    out[w] = scalar[w] * AT[w%L]            # broadcast, split DVE/ACT/GPSIMD

Tiles of the same phase are processed in PAIRS: one PSUM bank holds both
UT tiles ([128, 2, 143] fp32 = 1144B/partition), and each of the 4
contraction matmuls covers both tiles as a strided moving operand
(N=2x143=286). 8 pair-banks in flight give the PE a 16-tile runway, so
matmuls stay back-to-back (probe: pipelined MMs hit ~N/2.4 ns; stalled
ones pay (219+N)/1.2). Streamed PE columns drop 44% vs the natural
layout, the DVE dot drops FD 512 -> 143.

HBM traffic is bf16 both ways (33MB/core); input blocks are single ~2MB
fully-contiguous DMAs. Sharded across 8 cores along the window axis; halo
handled host-side by overlapping shards (no collectives).
"""

import numpy as np

import concourse.bacc as bacc
import concourse.bass as bass  # noqa: F401
import concourse.mybir as mybir
import concourse.tile as tile
from concourse.bass_utils import run_bass_kernel_spmd

N_CORES = 8
M_DIM = 512
L_DIM = 512
SEQ = 131072
CHUNK = 128  # windows per tile
BC = 16  # tiles per DMA block
KK = 143  # band extent per tile (128 + max_rank - 1)
COLS_DMA = BC * CHUNK + 16  # 2064 input cols loaded per block
COLS_ALLOC = 11 * CHUNK + 1024  # 2432: AP view bound for the last pair window

_NC_CACHE = {}
_LAST_RESULTS = None  # BassKernelResults of the most recent run (for test.py)


def build_nc(nblocks: int, rank: int) -> bass.Bass:
    f32 = mybir.dt.float32
    bf16 = mybir.dt.bfloat16
    ntiles = nblocks * BC
    ncols = ntiles * CHUNK + 16

    mult = mybir.AluOpType.mult
    copy_f = mybir.ActivationFunctionType.Copy

    nc = bacc.Bacc()
    # input, transposed: v_d[s, d, row] = vec[row, 128*s + d]
    v_d = nc.dram_tensor("v", [4, CHUNK, ncols], bf16, kind="ExternalInput")
    # stationary C.T slices: ct_d[ph, s] = C[128ph:128ph+128, 128s:128s+128].T
    ct_d = nc.dram_tensor("ct", [4, 4, CHUNK, CHUNK], bf16, kind="ExternalInput")
    bm_d = nc.dram_tensor("bm", [CHUNK, KK], bf16, kind="ExternalInput")
    a_d = nc.dram_tensor("amat", [4, CHUNK, M_DIM], bf16, kind="ExternalInput")
    o_d = nc.dram_tensor("o", [nblocks, BC, CHUNK, M_DIM], bf16, kind="ExternalOutput")

    with tile.TileContext(nc) as tc:
        with (
            tc.tile_pool(name="consts", bufs=1) as consts,
            tc.tile_pool(name="blocks", bufs=3) as blocks,
            tc.tile_pool(name="outs", bufs=2) as outs,
            tc.tile_pool(name="psump", bufs=8, space="PSUM") as psump,
            tc.tile_pool(name="work", bufs=8) as work,
        ):
            ct16 = consts.tile([CHUNK, 4, 4, CHUNK], bf16, tag="ct16")
            nc.sync.dma_start(out=ct16, in_=ct_d[:].transpose([2, 0, 1, 3]))
            bmt = consts.tile([CHUNK, KK], bf16, tag="bm")
            nc.sync.dma_start(out=bmt, in_=bm_d[:])
            a4 = consts.tile([CHUNK, 4, M_DIM], bf16, tag="a4")
            nc.sync.dma_start(out=a4, in_=a_d[:].transpose([1, 0, 2]))

            def load_block(b):
                vt = blocks.tile([CHUNK, 4, COLS_ALLOC], bf16, tag="vt")
                nc.sync.dma_start(
                    out=vt[:, :, 0:COLS_DMA],
                    in_=v_d[
                        :, :, b * BC * CHUNK : b * BC * CHUNK + COLS_DMA
                    ].transpose([1, 0, 2]),
                )
                return vt

            PF = 2
            vts = {b: load_block(b) for b in range(min(PF, nblocks))}
            for b in range(nblocks):
                if b + PF < nblocks:
                    vts[b + PF] = load_block(b + PF)
                vt = vts[b]
                ot = outs.tile([CHUNK, BC, M_DIM], bf16, tag="ot")
                # Tiles c, c+4, c+8, c+12 share phase ph = t%4. Process in
                # pairs (c, c+4): 4 accumulating matmuls, each streaming a
                # strided [2, 143] window pair (N=286) into one PSUM bank.
                for g in range(4):
                    ph = (b * BC + g) % 4
                    for half in range(2):
                        cb = g + 8 * half
                        ps = psump.tile([CHUNK, 2, KK], f32, tag="ps")
                        for s in range(4):
                            win = vt[:, s, cb * CHUNK : cb * CHUNK + 1024]
                            mv = win.rearrange("p (t x) -> p t x", t=2, x=512)[
                                :, :, 0:KK
                            ]
                            nc.tensor.matmul(
                                ps,
                                ct16[:, ph, s, :],
                                mv,
                                start=(s == 0),
                                stop=(s == 3),
                            )
                        for i in range(2):
                            c = cb + 4 * i
                            t = b * BC + c
                            sc = work.tile([CHUNK, KK], bf16, tag="sc")
                            sv = work.tile([CHUNK, 1], f32, tag="sv")
                            nc.vector.scalar_tensor_tensor(
                                out=sc,
                                in0=ps[:, i, :],
                                scalar=1.0,
                                in1=bmt,
                                op0=mult,
                                op1=mult,
                                accum_out=sv,
                            )
                            m7 = t % 7
                            if m7 < 2:
                                nc.vector.tensor_scalar(
                                    out=ot[:, c, :],
                                    in0=a4[:, ph, :],
                                    scalar1=sv,
                                    scalar2=None,
                                    op0=mult,
                                )
                            elif m7 < 5:
                                nc.scalar.activation(
                                    out=ot[:, c, :],
                                    in_=a4[:, ph, :],
                                    func=copy_f,
                                    scale=sv,
                                )
                            else:
                                nc.gpsimd.tensor_tensor(
                                    ot[:, c, :],
                                    a4[:, ph, :],
                                    sv.broadcast_to([CHUNK, M_DIM]),
                                    mult,
                                )
                nc.sync.dma_start(out=o_d[b].transpose([1, 0, 2]), in_=ot)
                del vts[b]

    nc.finalize()
    return nc


def _get_nc(nblocks: int, rank: int) -> bass.Bass:
    key = (nblocks, rank)
    if key not in _NC_CACHE:
        _NC_CACHE[key] = build_nc(nblocks, rank)
    return _NC_CACHE[key]


def kernel(vec_seq, M, Acoeff, Bbasis, rank):
    global _LAST_RESULTS
    import ml_dtypes

    bf = ml_dtypes.bfloat16
    vec_seq = np.asarray(vec_seq, dtype=np.float32)
    M = np.asarray(M, dtype=np.float32)
    Acoeff = np.asarray(Acoeff, dtype=np.float32)
    Bbasis = np.asarray(Bbasis, dtype=np.float32)
    r = int(rank)
    S, m = vec_seq.shape
    assert m == M_DIM and Bbasis.shape[0] == L_DIM
    assert 1 <= r <= 16  # band extent 127+r must fit KK=143

    W = S - r + 1
    nblocks = -(-W // (N_CORES * CHUNK * BC))
    ntiles = nblocks * BC
    nw = ntiles * CHUNK
    ncols = nw + 16

    # Transposed bf16 input, once for the full sequence: [512, S]
    vT = np.ascontiguousarray(vec_seq.astype(bf).T)

    C = ((Bbasis.astype(np.float64) @ M.astype(np.float64)) / r).astype(np.float32)
    # ct[ph, s] = C[128ph:128(ph+1), 128s:128(s+1)].T
    ct = np.ascontiguousarray(
        C.reshape(4, CHUNK, 4, CHUNK).transpose(0, 2, 3, 1)
    ).astype(bf)
    AT = np.ascontiguousarray(Acoeff.T).astype(np.float32)
    a4 = np.ascontiguousarray(AT.reshape(4, CHUNK, M_DIM)).astype(bf)
    # band mask: bm[w, k] = 1 iff w <= k <= w + r - 1
    bm = np.zeros((CHUNK, KK), dtype=np.float32)
    for w in range(CHUNK):
        bm[w, w : w + r] = 1
    bm = bm.astype(bf)

    nc = _get_nc(nblocks, r)

    in_maps = []
    for k in range(N_CORES):
        lo = k * nw
        hi = min(S, lo + ncols)
        sh = np.zeros((M_DIM, ncols), dtype=bf)
        if hi > lo:
            sh[:, : hi - lo] = vT[:, lo:hi]
        im = {
            "v": sh.reshape(4, CHUNK, ncols),
            "ct": ct,
            "bm": bm,
            "amat": a4,
        }
        in_maps.append(im)

    res = run_bass_kernel_spmd(nc, in_maps, core_ids=list(range(N_CORES)))
    _LAST_RESULTS = res
    out = np.concatenate(
        [res.results[k]["o"].reshape(nw, M_DIM) for k in range(N_CORES)], axis=0
    )
    return np.ascontiguousarray(out[:W].astype(np.float32))


# revision 6
# speedup vs baseline: 5.0108x; 1.0626x over previous
"""Trainium2 Bass kernel for nn_NumDualDescriptorAB (sliding-window descriptor).

Reference computation:
    X = vec_seq @ M.T                       # [S, m]
    T[w] = mean_{r<rank} X[w+r]             # sliding window mean, W = S-rank+1
    j = w % L
    scalar[w] = Bbasis[j] . T[w]
    out[w]    = Acoeff.T[j] * scalar[w]

Algebraic rewrite (matmul is linear, dot distributes over the window sum):
    C = Bbasis @ M / rank                   # [L, m]  tiny - host precompute
    scalar[w] = sum_{r} C[w%L] . v[w+r]
    out[w]    = Acoeff.T[w%L] * scalar[w]

v5 dataflow ("transposed-V"): the input is uploaded TRANSPOSED (host-side
transpose is free), so the PE contracts over the feature dim directly:

    UT[j, k] = C[ph*128+j] . v[row k]       # per 128-window tile, phase ph
    scalar[w] = sum_{k=w..w+r-1} UT[w, k]   # banded free-axis sum: ONE fused
                                            # DVE scalar_tensor_tensor with a
                                            # constant 0/1 band mask, FD=143
    out[w] = scalar[w] * AT[w%L]            # broadcast, split DVE/ACT/GPSIMD

Tiles of the same phase are processed in PAIRS: one PSUM bank holds both
UT tiles ([128, 2, 143] fp32 = 1144B/partition), and each of the 4
contraction matmuls covers both tiles as a strided moving operand
(N=2x143=286). 8 pair-banks in flight give the PE a 16-tile runway, so
matmuls stay back-to-back (probe: pipelined MMs hit ~N/2.4 ns; stalled
ones pay (219+N)/1.2). Streamed PE columns drop 44% vs the natural
layout, the DVE dot drops FD 512 -> 143.

HBM traffic is bf16 both ways (33MB/core); input blocks are single ~2MB
fully-contiguous DMAs. Sharded across 8 cores along the window axis; halo
handled host-side by overlapping shards (no collectives).
"""

import numpy as np

import concourse.bacc as bacc
import concourse.bass as bass  # noqa: F401
import concourse.mybir as mybir
import concourse.tile as tile
from concourse.bass_utils import run_bass_kernel_spmd

N_CORES = 8
M_DIM = 512
L_DIM = 512
SEQ = 131072
CHUNK = 128  # windows per tile
BC = 16  # tiles per DMA block
KK = 143  # band extent per tile (128 + max_rank - 1)
COLS_DMA = BC * CHUNK + 16  # 2064 input cols loaded per block
COLS_ALLOC = 11 * CHUNK + 1024  # 2432: AP view bound for the last pair window

_NC_CACHE = {}
_LAST_RESULTS = None  # BassKernelResults of the most recent run (for test.py)


def build_nc(nblocks: int, rank: int) -> bass.Bass:
    f32 = mybir.dt.float32
    bf16 = mybir.dt.bfloat16
    ntiles = nblocks * BC
    ncols = ntiles * CHUNK + 16

    mult = mybir.AluOpType.mult
    copy_f = mybir.ActivationFunctionType.Copy

    nc = bacc.Bacc()
    # input, transposed: v_d[s, d, row] = vec[row, 128*s + d]
    v_d = nc.dram_tensor("v", [4, CHUNK, ncols], bf16, kind="ExternalInput")
    # stationary C.T slices: ct_d[ph, s] = C[128ph:128ph+128, 128s:128s+128].T
    ct_d = nc.dram_tensor("ct", [4, 4, CHUNK, CHUNK], bf16, kind="ExternalInput")
    bm_d = nc.dram_tensor("bm", [CHUNK, KK], bf16, kind="ExternalInput")
    a_d = nc.dram_tensor("amat", [4, CHUNK, M_DIM], bf16, kind="ExternalInput")
    # output kept in device-friendly [p, c, d] order per block (fully
    # contiguous 8KB half-block partition lines); host un-permutes.
    o_d = nc.dram_tensor("o", [nblocks, CHUNK, BC, M_DIM], bf16, kind="ExternalOutput")

    with tile.TileContext(nc) as tc:
        with (
            tc.tile_pool(name="consts", bufs=1) as consts,
            tc.tile_pool(name="blocks", bufs=3) as blocks,
            tc.tile_pool(name="outs", bufs=2) as outs,
            tc.tile_pool(name="psump", bufs=8, space="PSUM") as psump,
            tc.tile_pool(name="work", bufs=8) as work,
        ):
            ct16 = consts.tile([CHUNK, 4, 4, CHUNK], bf16, tag="ct16")
            nc.sync.dma_start(out=ct16, in_=ct_d[:].transpose([2, 0, 1, 3]))
            bmt = consts.tile([CHUNK, KK], bf16, tag="bm")
            nc.sync.dma_start(out=bmt, in_=bm_d[:])
            a4 = consts.tile([CHUNK, 4, M_DIM], bf16, tag="a4")
            nc.sync.dma_start(out=a4, in_=a_d[:].transpose([1, 0, 2]))

            def load_block(b):
                vt = blocks.tile([CHUNK, 4, COLS_ALLOC], bf16, tag="vt")
                nc.sync.dma_start(
                    out=vt[:, :, 0:COLS_DMA],
                    in_=v_d[
                        :, :, b * BC * CHUNK : b * BC * CHUNK + COLS_DMA
                    ].transpose([1, 0, 2]),
                )
                return vt

            PF = 2
            vts = {b: load_block(b) for b in range(min(PF, nblocks))}
            for b in range(nblocks):
                if b + PF < nblocks:
                    vts[b + PF] = load_block(b + PF)
                vt = vts[b]
                ot = outs.tile([CHUNK, BC, M_DIM], bf16, tag="ot")
                # Tiles c, c+4, c+8, c+12 share phase ph = t%4. Process in
                # pairs (c, c+4): 4 accumulating matmuls, each streaming a
                # strided [2, 143] window pair (N=286) into one PSUM bank.
                # Half-block order (tiles 0-7 then 8-15) lets each half's
                # output store issue early on the ACT HWDGE ring.
                for half in range(2):
                    for g in range(4):
                        cb = g + 8 * half
                        ph = (b * BC + cb) % 4
                        ps = psump.tile([CHUNK, 2, KK], f32, tag="ps")
                        for s in range(4):
                            win = vt[:, s, cb * CHUNK : cb * CHUNK + 1024]
                            mv = win.rearrange("p (t x) -> p t x", t=2, x=512)[
                                :, :, 0:KK
                            ]
                            nc.tensor.matmul(
                                ps,
                                ct16[:, ph, s, :],
                                mv,
                                start=(s == 0),
                                stop=(s == 3),
                            )
                        for i in range(2):
                            c = cb + 4 * i
                            t = b * BC + c
                            sc = work.tile([CHUNK, KK], bf16, tag="sc")
                            sv = work.tile([CHUNK, 1], f32, tag="sv")
                            nc.vector.scalar_tensor_tensor(
                                out=sc,
                                in0=ps[:, i, :],
                                scalar=1.0,
                                in1=bmt,
                                op0=mult,
                                op1=mult,
                                accum_out=sv,
                            )
                            m7 = t % 7
                            if m7 < 2:
                                nc.vector.tensor_scalar(
                                    out=ot[:, c, :],
                                    in0=a4[:, ph, :],
                                    scalar1=sv,
                                    scalar2=None,
                                    op0=mult,
                                )
                            elif m7 < 5:
                                nc.scalar.activation(
                                    out=ot[:, c, :],
                                    in_=a4[:, ph, :],
                                    func=copy_f,
                                    scale=sv,
                                )
                            else:
                                nc.gpsimd.tensor_tensor(
                                    ot[:, c, :],
                                    a4[:, ph, :],
                                    sv.broadcast_to([CHUNK, M_DIM]),
                                    mult,
                                )
                    nc.scalar.dma_start(
                        out=o_d[b, :, half * 8 : half * 8 + 8, :],
                        in_=ot[:, half * 8 : half * 8 + 8, :],
                    )
                del vts[b]

    nc.finalize()
    return nc


def _get_nc(nblocks: int, rank: int) -> bass.Bass:
    key = (nblocks, rank)
    if key not in _NC_CACHE:
        _NC_CACHE[key] = build_nc(nblocks, rank)
    return _NC_CACHE[key]


def kernel(vec_seq, M, Acoeff, Bbasis, rank):
    global _LAST_RESULTS
    import ml_dtypes

    bf = ml_dtypes.bfloat16
    vec_seq = np.asarray(vec_seq, dtype=np.float32)
    M = np.asarray(M, dtype=np.float32)
    Acoeff = np.asarray(Acoeff, dtype=np.float32)
    Bbasis = np.asarray(Bbasis, dtype=np.float32)
    r = int(rank)
    S, m = vec_seq.shape
    assert m == M_DIM and Bbasis.shape[0] == L_DIM
    assert 1 <= r <= 16  # band extent 127+r must fit KK=143

    W = S - r + 1
    nblocks = -(-W // (N_CORES * CHUNK * BC))
    ntiles = nblocks * BC
    nw = ntiles * CHUNK
    ncols = nw + 16

    # Transposed bf16 input, once for the full sequence: [512, S]
    vT = np.ascontiguousarray(vec_seq.astype(bf).T)

    C = ((Bbasis.astype(np.float64) @ M.astype(np.float64)) / r).astype(np.float32)
    # ct[ph, s] = C[128ph:128(ph+1), 128s:128(s+1)].T
    ct = np.ascontiguousarray(
        C.reshape(4, CHUNK, 4, CHUNK).transpose(0, 2, 3, 1)
    ).astype(bf)
    AT = np.ascontiguousarray(Acoeff.T).astype(np.float32)
    a4 = np.ascontiguousarray(AT.reshape(4, CHUNK, M_DIM)).astype(bf)
    # band mask: bm[w, k] = 1 iff w <= k <= w + r - 1
    bm = np.zeros((CHUNK, KK), dtype=np.float32)
    for w in range(CHUNK):
        bm[w, w : w + r] = 1
    bm = bm.astype(bf)

    nc = _get_nc(nblocks, r)

    in_maps = []
    for k in range(N_CORES):
        lo = k * nw
        hi = min(S, lo + ncols)
        sh = np.zeros((M_DIM, ncols), dtype=bf)
        if hi > lo:
            sh[:, : hi - lo] = vT[:, lo:hi]
        im = {
            "v": sh.reshape(4, CHUNK, ncols),
            "ct": ct,
            "bm": bm,
            "amat": a4,
        }
        in_maps.append(im)

    res = run_bass_kernel_spmd(nc, in_maps, core_ids=list(range(N_CORES)))
    _LAST_RESULTS = res
    out = np.concatenate(
        [
            res.results[k]["o"]
            .reshape(nblocks, CHUNK, BC, M_DIM)
            .transpose(0, 2, 1, 3)
            .reshape(nw, M_DIM)
            for k in range(N_CORES)
        ],
        axis=0,
    )
    return np.ascontiguousarray(out[:W].astype(np.float32))
